# revision 1
# baseline (speedup 1.0000x reference)
"""Batched GATv2 attention kernel for 8 Trainium2 NeuronCores.

Data-parallel: one graph (batch element) per core.

Math (per graph):
  xl = x@W_l + b_l, xr = x@W_r + b_r   (reshape [N, H=4, C=32], HC=128)
  e[i,j,h] = sum_c att[h,c] * LeakyReLU_0.2(xr[i,hc] + xl[j,hc])
           = 0.2*(er[i,h] + el[j,h]) + 0.8*sum_c att[h,c]*relu(xr+xl)
  alpha = softmax_j(e + mask);  out[i] = sum_j alpha[i,j,h]*xl[j,hc] + bias
The er term is constant over j -> cancels in softmax -> dropped.
0.2*el[h,j] and the adjacency mask are folded into a host-built `maskel`
tensor (fp32, exact); 0.8*att into the pack-matmul stationaries.

On-chip pipeline (per core), hot loop over 16 groups g of 32 source nodes j.
Group row packing r in [0,128): q=r//32, s'=(r%32)//4, h=r%4, j(r)=32g+8q+s'.
  S'_j [128 hc, 512 i] = relu(xrT + xlT[:,j])  (bf16; whole-j split between
      ScalarE Relu-with-bias and VectorE tensor_scalar add+max at 4x)
  E_g[32q+4s'+h, i] += 0.4*att-dot         (PE bf16; 8 shifted stationaries;
      sp-outer/q-inner order so the 4 col-group MMs run concurrently)
  E_g += maskel_g (fp32); aU_g = exp(E_g)  (VectorE add, ScalarE Exp -> bf16)
  xlexp_g[r, hc] = xl[j(r), hc]*head_mask  (PE select-matmul + VectorE)
  numT [128 hc, 512 i] += xlexp_g.T @ aU_g (PE, PSUM accum over g)
  den  [4 h, 512 i]    += hsel.T @ aU_g    (PE)
Host: out[i, hc] = numT[hc, i] / den[hc//32, i] + bias[hc].
"""
import numpy as np

B, N, IN_DIM, HEADS, PER_HEAD = 8, 512, 256, 4, 32
OUT_DIM = HEADS * PER_HEAD  # 128
HC = 128
NEG = -1.0e30
NGRP = 16           # j-groups per graph
ACT_EVERY = 6       # every 6th j's S' computed on ScalarE, rest on VectorE

_prog_cache = {}


# ----------------------------------------------------------------- host prep
def _row_maps():
    r = np.arange(128)
    return r // 32, (r % 32) // 4, r % 4      # q, s', h


def _bf16(a):
    import ml_dtypes
    return np.asarray(a, np.float32).astype(ml_dtypes.bfloat16)


def _host_consts(att):
    q, sp, h = _row_maps()
    att = np.asarray(att, np.float32)
    att_e8 = np.zeros((128, 8 * 32), np.float32)
    for s in range(8):
        for hh in range(HEADS):
            for c in range(PER_HEAD):
                att_e8[hh * 32 + c, 32 * s + 4 * s + hh] = 0.8 * att[hh, c]
    sel = np.zeros((128, 4 * 128), np.float32)
    r = np.arange(128)
    for v in range(4):
        jl = 32 * v + 8 * q + sp
        sel[jl, 128 * v + r] = 1.0
    hsel = np.zeros((128, 4), np.float32)
    hsel[r, h] = 1.0
    hmask = np.zeros((128, 128), np.float32)
    for rr in range(128):
        hmask[rr, (rr % 4) * 32:(rr % 4 + 1) * 32] = 1.0
    return _bf16(att_e8), _bf16(sel), _bf16(hsel), _bf16(hmask)


def _host_prep_core(b, x, adj, W_l, b_l, att):
    q, sp, h = _row_maps()
    xb = np.asarray(x[b], np.float32)
    xT = np.ascontiguousarray(xb.T)                       # [256, 512] fp32
    A = np.asarray(adj[b]).copy()
    np.fill_diagonal(A, 1)
    m = (A.T != 0)                                        # m[i, j]
    xl_host = xb @ np.asarray(W_l, np.float32) + np.asarray(b_l, np.float32)
    el = np.einsum('hc,jhc->hj', np.asarray(att, np.float32),
                   xl_host.reshape(N, HEADS, PER_HEAD))   # [4, 512]
    maskel = np.empty((NGRP * 128, N), np.float32)
    for g in range(NGRP):
        j = 32 * g + 8 * q + sp
        mb = np.where(m[:, j].T, 0.0, NEG)                # [128 r, 512 i]
        maskel[128 * g:128 * (g + 1)] = mb + (0.2 * el[h, j])[:, None]
    return xT, maskel


# -------------------------------------------------------------- bass program
def _build_program(repeat=1):
    from contextlib import ExitStack
    import concourse.bass as bass
    import concourse.tile as tile
    import concourse.mybir as mybir
    from concourse import bacc

    f32 = mybir.dt.float32
    bf16 = mybir.dt.bfloat16
    ADD = mybir.AluOpType.add
    MULT = mybir.AluOpType.mult
    MAX = mybir.AluOpType.max
    RELU = mybir.ActivationFunctionType.Relu
    EXP = mybir.ActivationFunctionType.Exp

    nc = bacc.Bacc("TRN2", target_bir_lowering=False, debug=False,
                   num_devices=8)

    def din(name, shape, dt=f32):
        return nc.dram_tensor(name, shape, dt, kind="ExternalInput").ap()

    xT_d = din("xT", [IN_DIM, N])
    maskel_d = din("maskel", [NGRP * 128, N])
    Wl_d = din("W_l_bf", [IN_DIM, HC], bf16)
    Wr_d = din("W_r_bf", [IN_DIM, HC], bf16)
    blc_d = din("bl_col", [HC, 1])
    brc_d = din("br_col", [HC, 1])
    blr_d = din("bl_row", [1, HC], bf16)
    brr_d = din("br_row", [1, HC], bf16)
    att8_d = din("att_e8", [128, 256], bf16)
    sel_d = din("sel", [128, 512], bf16)
    hsel_d = din("hsel", [128, 4], bf16)
    hmask_d = din("hmask", [128, 128], bf16)
    numT_d = nc.dram_tensor("numT", [HC, N], f32, kind="ExternalOutput").ap()
    den_d = nc.dram_tensor("den", [HEADS, N], f32, kind="ExternalOutput").ap()

    with tile.TileContext(nc) as tc, ExitStack() as ctx:
        const = ctx.enter_context(tc.tile_pool(name="const", bufs=1))
        mpool = ctx.enter_context(tc.tile_pool(name="maskel", bufs=3))
        spool = ctx.enter_context(tc.tile_pool(name="S", bufs=8))
        aupool = ctx.enter_context(tc.tile_pool(name="aU", bufs=3))
        xpool = ctx.enter_context(tc.tile_pool(name="xlexp", bufs=2))
        psE = ctx.enter_context(tc.tile_pool(name="psE", bufs=2, space="PSUM"))
        psP = ctx.enter_context(tc.tile_pool(name="psP", bufs=1, space="PSUM"))
        psN = ctx.enter_context(tc.tile_pool(name="psN", bufs=1, space="PSUM"))
        psD = ctx.enter_context(tc.tile_pool(name="psD", bufs=1, space="PSUM"))
        psR = ctx.enter_context(tc.tile_pool(name="psR", bufs=2, space="PSUM"))

        # ---- load constants/inputs
        xT = const.tile([128, 2 * N], f32)        # two k-tiles side by side
        for kt in range(2):
            nc.sync.dma_start(out=xT[:, N * kt:N * (kt + 1)],
                              in_=xT_d[128 * kt:128 * (kt + 1), :])
        Wl = const.tile([128, 256], bf16)
        Wr = const.tile([128, 256], bf16)
        for kt in range(2):
            nc.sync.dma_start(out=Wl[:, 128 * kt:128 * (kt + 1)],
                              in_=Wl_d[128 * kt:128 * (kt + 1), :])
            nc.sync.dma_start(out=Wr[:, 128 * kt:128 * (kt + 1)],
                              in_=Wr_d[128 * kt:128 * (kt + 1), :])
        blc = const.tile([128, 1], f32)
        brc = const.tile([128, 1], f32)
        blr = const.tile([1, 128], bf16)
        brr = const.tile([1, 128], bf16)
        nc.sync.dma_start(out=blc[:], in_=blc_d[:])
        nc.sync.dma_start(out=brc[:], in_=brc_d[:])
        nc.sync.dma_start(out=blr[:], in_=blr_d[:])
        nc.sync.dma_start(out=brr[:], in_=brr_d[:])
        att8 = const.tile([128, 256], bf16)
        nc.sync.dma_start(out=att8[:], in_=att8_d[:])
        sel = const.tile([128, 512], bf16)
        nc.sync.dma_start(out=sel[:], in_=sel_d[:])
        hsel = const.tile([128, 4], bf16)
        nc.sync.dma_start(out=hsel[:], in_=hsel_d[:])
        hmask = const.tile([128, 128], bf16)
        nc.sync.dma_start(out=hmask[:], in_=hmask_d[:])
        ones = const.tile([1, N], bf16)
        nc.vector.memset(ones[:], 1.0)
        xT_bf = const.tile([128, 2 * N], bf16)
        nc.vector.tensor_copy(xT_bf[:], xT[:])

        # ---- projections (bf16 operands, fp32 PSUM accumulation)
        xlT = const.tile([128, N], f32)
        xrT = const.tile([128, N], f32)
        xlT_bf = const.tile([128, N], bf16)
        xrT_bf = const.tile([128, N], bf16)
        xl_bf = const.tile([128, 512], bf16)      # [j-local, 128*t + hc]
        for (W, brow, dstT, dstTbf) in ((Wl, blr, xlT, xlT_bf),
                                        (Wr, brr, xrT, xrT_bf)):
            ps = psP.tile([128, N], f32, tag="proj")
            nc.tensor.matmul(ps[:], W[:, 0:128], xT_bf[:, 0:N],
                             start=True, stop=False)
            nc.tensor.matmul(ps[:], W[:, 128:256], xT_bf[:, N:2 * N],
                             start=False, stop=False)
            nc.tensor.matmul(ps[:], brow[0:1, :], ones[0:1, :],
                             start=False, stop=True)
            nc.vector.tensor_copy(dstT[:], ps[:])
            nc.vector.tensor_copy(dstTbf[:], ps[:])
        for t in range(4):
            ps = psP.tile([128, 128], f32, tag="proj")
            nc.tensor.matmul(ps[:], xT_bf[:, 128 * t:128 * (t + 1)],
                             Wl[:, 0:128], start=True, stop=False)
            nc.tensor.matmul(ps[:], xT_bf[:, N + 128 * t:N + 128 * (t + 1)],
                             Wl[:, 128:256], start=False, stop=False)
            nc.tensor.matmul(ps[:], ones[0:1, 0:128], blr[0:1, :],
                             start=False, stop=True)
            nc.vector.tensor_copy(xl_bf[:, 128 * t:128 * (t + 1)], ps[:])

        # ---- hot loop
        numT_ps = psN.tile([128, N], f32)
        den_ps = psD.tile([128, N], f32)
        for g in [g for _ in range(repeat) for g in range(NGRP)]:
            mk = mpool.tile([128, N], f32, tag="mk")
            nc.sync.dma_start(out=mk[:], in_=maskel_d[128 * g:128 * (g + 1), :])
            Eg = psE.tile([128, N], f32, tag="Eg")
            for sp in range(8):
                Ss = []
                for q in range(4):
                    j = 32 * g + 8 * q + sp
                    S = spool.tile([128, N], bf16, tag="S")
                    if j % ACT_EVERY == 0:
                        nc.scalar.activation(S[:], xrT[:], RELU,
                                             bias=xlT[:, j:j + 1])
                    else:
                        nc.vector.tensor_scalar(S[:], xrT_bf[:],
                                                xlT[:, j:j + 1], 0.0,
                                                ADD, MAX)
                    Ss.append(S)
                for q in range(4):
                    nc.tensor.matmul(Eg[32 * q:32 * (q + 1), :],
                                     att8[:, 32 * sp:32 * (sp + 1)],
                                     Ss[q][:],
                                     start=(sp == 0), stop=(sp == 7),
                                     tile_position=(0, 32 * q),
                                     skip_group_check=True)
            nc.vector.tensor_tensor(Eg[:], Eg[:], mk[:], ADD)
            aU = aupool.tile([128, N], bf16, tag="aU")
            nc.scalar.activation(aU[:], Eg[:], EXP)
            v, t = g % 4, g // 4
            rep = psR.tile([128, 128], f32, tag="rep")
            nc.tensor.matmul(rep[:], sel[:, 128 * v:128 * (v + 1)],
                             xl_bf[:, 128 * t:128 * (t + 1)],
                             start=True, stop=True)
            xlexp = xpool.tile([128, 128], bf16, tag="xlexp")
            nc.vector.tensor_tensor(xlexp[:], rep[:], hmask[:], MULT)
            nc.tensor.matmul(numT_ps[:], xlexp[:], aU[:],
                             start=(g == 0), stop=(g == NGRP - 1),
                             skip_group_check=True)
            nc.tensor.matmul(den_ps[0:HEADS, :], hsel[:], aU[:],
                             start=(g == 0), stop=(g == NGRP - 1),
                             skip_group_check=True)

        # ---- outputs
        numT_sb = const.tile([128, N], f32)
        den_sb = const.tile([HEADS, N], f32)
        nc.vector.tensor_copy(numT_sb[:], numT_ps[:])
        nc.vector.tensor_copy(den_sb[:], den_ps[0:HEADS, :])
        nc.sync.dma_start(out=numT_d[:], in_=numT_sb[:])
        nc.sync.dma_start(out=den_d[:], in_=den_sb[:])

    nc.compile()
    return nc


def _get_program(repeat=1):
    key = ("nc", repeat)
    if key not in _prog_cache:
        _prog_cache[key] = _build_program(repeat)
    return _prog_cache[key]


def _make_in_maps(x, adj, W_l, b_l, W_r, b_r, att):
    att_e8, sel, hsel, hmask = _host_consts(att)
    shared = {
        "W_l_bf": _bf16(W_l), "W_r_bf": _bf16(W_r),
        "bl_col": np.ascontiguousarray(np.asarray(b_l, np.float32).reshape(HC, 1)),
        "br_col": np.ascontiguousarray(np.asarray(b_r, np.float32).reshape(HC, 1)),
        "bl_row": np.ascontiguousarray(_bf16(b_l).reshape(1, HC)),
        "br_row": np.ascontiguousarray(_bf16(b_r).reshape(1, HC)),
        "att_e8": att_e8, "sel": sel, "hsel": hsel, "hmask": hmask,
    }
    in_maps = []
    for b in range(B):
        xT, maskel = _host_prep_core(b, x, adj, W_l, b_l, att)
        in_maps.append({"xT": xT, "maskel": maskel, **shared})
    return in_maps


# ------------------------------------------------------------------- kernel
def kernel(x, adj, W_l, b_l, W_r, b_r, att, bias):
    from concourse.bass_utils import run_bass_kernel_spmd

    x = np.asarray(x, np.float32)
    adj = np.asarray(adj)
    bias = np.asarray(bias, np.float32)

    in_maps = _make_in_maps(x, adj, W_l, b_l, W_r, b_r, att)
    nc = _get_program()
    res = run_bass_kernel_spmd(nc, in_maps, list(range(B)))

    out = np.empty((B, N, OUT_DIM), np.float32)
    for b in range(B):
        numT = np.asarray(res.results[b]["numT"])   # [128, 512]
        den = np.asarray(res.results[b]["den"])     # [4, 512]
        denx = np.repeat(den.T, PER_HEAD, axis=1)   # [512, 128]
        out[b] = numT.T / denx + bias
    return out



# revision 2
# speedup vs baseline: 2389.5108x; 2389.5108x over previous
"""Batched GATv2 attention kernel for 8 Trainium2 NeuronCores.

Data-parallel: one graph (batch element) per core.

Math (per graph):
  xl = x@W_l + b_l, xr = x@W_r + b_r   (reshape [N, H=4, C=32], HC=128)
  e[i,j,h] = sum_c att[h,c] * LeakyReLU_0.2(xr[i,hc] + xl[j,hc])
           = 0.2*(er[i,h] + el[j,h]) + 0.8*sum_c att[h,c]*relu(xr+xl)
  alpha = softmax_j(e + mask);  out[i] = sum_j alpha[i,j,h]*xl[j,hc] + bias
The er term is constant over j -> cancels in softmax -> dropped.
0.2*el[h,j] and the adjacency mask are folded into a host-built `maskel`
tensor (bf16); 0.8*att into the pack-matmul stationaries.

On-chip pipeline (per core), hot loop over 16 groups g of 32 source nodes j.
Group row packing r in [0,128): q=r//32, s'=(r%32)//4, h=r%4, j(r)=32g+8q+s'.
  S'_j [128 hc, 512 i] = relu(xrT + xlT[:,j])  (bf16; 24 of 32 j-slots per
      group on VectorE tensor_scalar add+max at 4x, 8 on ScalarE Relu+bias)
  E_g[32q+4s'+h, i] += 0.4*att-dot         (PE bf16; 8 shifted stationaries;
      4 col-tiled MMs per subset run concurrently)
  E_g += maskel_g  (PE: identity-stationary matmul, same PSUM accum group;
      maskel tiles preloaded to SBUF in the preamble -> zero hot-loop DMA)
  aU_g = exp(E_g)                          (ScalarE Exp, PSUM->SBUF, bf16)
  numT [128 hc, 512 i] += xlexp_g.T @ aU_g (PE, PSUM accum; xlexp = head-
      masked row-gathered xl, precomputed on host, preloaded to SBUF)
  den  [4 h, 512 i]    += hsel.T @ aU_g    (PE)
Software pipelining: group body emits S+att+mask(g), exp(g-1), num/den(g-2)
so the PE never head-blocks on the ScalarE exp round trip.
Host: out[i, hc] = numT[hc, i] / den[hc//32, i] + bias[hc].
"""
import numpy as np

B, N, IN_DIM, HEADS, PER_HEAD = 8, 512, 256, 4, 32
OUT_DIM = HEADS * PER_HEAD  # 128
HC = 128
NEG = -1.0e30
NGRP = 16           # j-groups per graph
ND, NA = 24, 8      # S'-op split per group: 24 on VectorE, 8 on ScalarE

_prog_cache = {}


# ----------------------------------------------------------------- host prep
def _row_maps():
    r = np.arange(128)
    return r // 32, (r % 32) // 4, r % 4      # q, s', h


def _bf16(a):
    import ml_dtypes
    return np.asarray(a, np.float32).astype(ml_dtypes.bfloat16)


def _host_consts(att):
    att = np.asarray(att, np.float32)
    att_e8 = np.zeros((128, 8 * 32), np.float32)
    for s in range(8):
        for hh in range(HEADS):
            for c in range(PER_HEAD):
                att_e8[hh * 32 + c, 32 * s + 4 * s + hh] = 0.8 * att[hh, c]
    r = np.arange(128)
    hsel = np.zeros((128, 4), np.float32)
    hsel[r, r % 4] = 1.0
    ident = np.eye(128, dtype=np.float32)
    return _bf16(att_e8), _bf16(hsel), _bf16(ident)


def _host_prep_core(b, x, adj, W_l, b_l, att):
    q, sp, h = _row_maps()
    xb = np.asarray(x[b], np.float32)
    xT = np.ascontiguousarray(xb.T)                       # [256, 512] fp32
    A = np.asarray(adj[b]).copy()
    np.fill_diagonal(A, 1)
    m = (A.T != 0)                                        # m[i, j]
    xl_host = xb @ np.asarray(W_l, np.float32) + np.asarray(b_l, np.float32)
    el = np.einsum('hc,jhc->hj', np.asarray(att, np.float32),
                   xl_host.reshape(N, HEADS, PER_HEAD))   # [4, 512]
    maskel = np.empty((NGRP * 128, N), np.float32)
    xlexp = np.zeros((NGRP * 128, HC), np.float32)
    hc = np.arange(HC)
    for g in range(NGRP):
        j = 32 * g + 8 * q + sp
        mb = np.where(m[:, j].T, 0.0, NEG)                # [128 r, 512 i]
        maskel[128 * g:128 * (g + 1)] = mb + (0.2 * el[h, j])[:, None]
        xe = xl_host[j] * (hc[None, :] // 32 == h[:, None])
        xlexp[128 * g:128 * (g + 1)] = xe
    return xT, _bf16(maskel), _bf16(xlexp)


def _assign_pattern(nd=ND, na=NA):
    """Evenly interleaved 32-slot engine pattern ('v' / 'a')."""
    slots = []
    cnt = {"v": 0, "a": 0}
    tgt = {"v": nd, "a": na}
    for _ in range(32):
        k = min(cnt, key=lambda e: (cnt[e] + 1) / tgt[e] if tgt[e] else 1e18)
        slots.append(k)
        cnt[k] += 1
    return slots


# -------------------------------------------------------------- bass program
def _build_program(repeat=1):
    from contextlib import ExitStack
    import concourse.tile as tile
    import concourse.mybir as mybir
    from concourse import bacc

    f32 = mybir.dt.float32
    bf16 = mybir.dt.bfloat16
    ADD = mybir.AluOpType.add
    MAX = mybir.AluOpType.max
    RELU = mybir.ActivationFunctionType.Relu
    EXP = mybir.ActivationFunctionType.Exp
    assign = _assign_pattern()

    nc = bacc.Bacc("TRN2", target_bir_lowering=False, debug=False,
                   num_devices=8)

    def din(name, shape, dt=f32):
        return nc.dram_tensor(name, shape, dt, kind="ExternalInput").ap()

    xT_d = din("xT", [IN_DIM, N])
    maskel_d = din("maskel_bf", [NGRP * 128, N], bf16)
    xlexp_d = din("xlexp", [NGRP * 128, HC], bf16)
    Wl_d = din("W_l_bf", [IN_DIM, HC], bf16)
    Wr_d = din("W_r_bf", [IN_DIM, HC], bf16)
    blr_d = din("bl_row", [1, HC], bf16)
    brr_d = din("br_row", [1, HC], bf16)
    att8_d = din("att_e8", [128, 256], bf16)
    hsel_d = din("hsel", [128, 4], bf16)
    ident_d = din("ident", [128, 128], bf16)
    numT_d = nc.dram_tensor("numT", [HC, N], f32, kind="ExternalOutput").ap()
    den_d = nc.dram_tensor("den", [HEADS, N], f32, kind="ExternalOutput").ap()

    with tile.TileContext(nc) as tc, ExitStack() as ctx:
        const = ctx.enter_context(tc.tile_pool(name="const", bufs=1))
        spool = ctx.enter_context(tc.tile_pool(name="S", bufs=12))
        aupool = ctx.enter_context(tc.tile_pool(name="aU", bufs=3))
        psE = ctx.enter_context(tc.tile_pool(name="psE", bufs=3, space="PSUM"))
        psP = ctx.enter_context(tc.tile_pool(name="psP", bufs=1, space="PSUM"))
        psN = ctx.enter_context(tc.tile_pool(name="psN", bufs=1, space="PSUM"))
        psD = ctx.enter_context(tc.tile_pool(name="psD", bufs=1, space="PSUM"))

        # ---- load constants/inputs (one-time; outside the hot loop)
        xT = const.tile([128, 2 * N], f32)
        for kt in range(2):
            nc.sync.dma_start(out=xT[:, N * kt:N * (kt + 1)],
                              in_=xT_d[128 * kt:128 * (kt + 1), :])
        Wl = const.tile([128, 256], bf16)
        Wr = const.tile([128, 256], bf16)
        for kt in range(2):
            nc.sync.dma_start(out=Wl[:, 128 * kt:128 * (kt + 1)],
                              in_=Wl_d[128 * kt:128 * (kt + 1), :])
            nc.sync.dma_start(out=Wr[:, 128 * kt:128 * (kt + 1)],
                              in_=Wr_d[128 * kt:128 * (kt + 1), :])
        blr = const.tile([1, 128], bf16)
        brr = const.tile([1, 128], bf16)
        nc.sync.dma_start(out=blr[:], in_=blr_d[:])
        nc.sync.dma_start(out=brr[:], in_=brr_d[:])
        att8 = const.tile([128, 256], bf16)
        nc.sync.dma_start(out=att8[:], in_=att8_d[:])
        hsel = const.tile([128, 4], bf16)
        nc.sync.dma_start(out=hsel[:], in_=hsel_d[:])
        ident = const.tile([128, 128], bf16)
        nc.sync.dma_start(out=ident[:], in_=ident_d[:])
        mks = []
        xes = []
        for g in range(NGRP):
            mk = const.tile([128, N], bf16, tag=f"mk{g}")
            nc.sync.dma_start(out=mk[:], in_=maskel_d[128 * g:128 * (g + 1), :])
            mks.append(mk)
            xe = const.tile([128, HC], bf16, tag=f"xe{g}")
            nc.sync.dma_start(out=xe[:], in_=xlexp_d[128 * g:128 * (g + 1), :])
            xes.append(xe)
        ones = const.tile([1, N], bf16)
        nc.vector.memset(ones[:], 1.0)
        xT_bf = const.tile([128, 2 * N], bf16)
        nc.vector.tensor_copy(xT_bf[:], xT[:])

        # ---- projections (bf16 operands, fp32 PSUM accumulation)
        xlT = const.tile([128, N], f32)
        xrT = const.tile([128, N], f32)
        xrT_bf = const.tile([128, N], bf16)
        for (W, brow, dstT, dstTbf) in ((Wl, blr, xlT, None),
                                        (Wr, brr, xrT, xrT_bf)):
            ps = psP.tile([128, N], f32, tag="proj")
            nc.tensor.matmul(ps[:], W[:, 0:128], xT_bf[:, 0:N],
                             start=True, stop=False)
            nc.tensor.matmul(ps[:], W[:, 128:256], xT_bf[:, N:2 * N],
                             start=False, stop=False)
            nc.tensor.matmul(ps[:], brow[0:1, :], ones[0:1, :],
                             start=False, stop=True)
            nc.vector.tensor_copy(dstT[:], ps[:])
            if dstTbf is not None:
                nc.vector.tensor_copy(dstTbf[:], ps[:])

        # ---- hot loop (software-pipelined: exp lags 1 group, accum lags 2)
        numT_ps = psN.tile([128, N], f32)
        den_ps = psD.tile([128, N], f32)
        glist = [g for _ in range(repeat) for g in range(NGRP)]
        total = len(glist)
        Egs = {}
        aUs = {}

        def emit_S_att(it):
            g = glist[it]
            Eg = psE.tile([128, N], f32, tag="Eg")
            Egs[it] = Eg
            for sp in range(8):
                Ss = []
                for q in range(4):
                    j = 32 * g + 8 * q + sp
                    S = spool.tile([128, N], bf16, tag="S")
                    if assign[4 * sp + q] == "a":
                        nc.scalar.activation(S[:], xrT[:], RELU,
                                             bias=xlT[:, j:j + 1])
                    else:
                        nc.vector.tensor_scalar(S[:], xrT_bf[:],
                                                xlT[:, j:j + 1], 0.0,
                                                ADD, MAX)
                    Ss.append(S)
                for q in range(4):
                    nc.tensor.matmul(Eg[32 * q:32 * (q + 1), :],
                                     att8[:, 32 * sp:32 * (sp + 1)],
                                     Ss[q][:],
                                     start=(sp == 0), stop=False,
                                     tile_position=(0, 32 * q),
                                     skip_group_check=True)
            nc.tensor.matmul(Eg[:], ident[:], mks[g][:],
                             start=False, stop=True,
                             skip_group_check=True)

        def emit_exp(it):
            aU = aupool.tile([128, N], bf16, tag="aU")
            nc.scalar.activation(aU[:], Egs.pop(it)[:], EXP)
            aUs[it] = aU

        def emit_acc(it):
            g = glist[it]
            first = it % NGRP == 0
            last = it % NGRP == NGRP - 1
            aU = aUs.pop(it)
            nc.tensor.matmul(numT_ps[:], xes[g][:], aU[:],
                             start=first, stop=last,
                             skip_group_check=True)
            nc.tensor.matmul(den_ps[0:HEADS, :], hsel[:], aU[:],
                             start=first, stop=last,
                             skip_group_check=True)

        for it in range(total):
            emit_S_att(it)
            if it >= 1:
                emit_exp(it - 1)
            if it >= 2:
                emit_acc(it - 2)
        emit_exp(total - 1)
        emit_acc(total - 2)
        emit_acc(total - 1)

        # ---- outputs
        numT_sb = const.tile([128, N], f32)
        den_sb = const.tile([HEADS, N], f32)
        nc.vector.tensor_copy(numT_sb[:], numT_ps[:])
        nc.vector.tensor_copy(den_sb[:], den_ps[0:HEADS, :])
        nc.sync.dma_start(out=numT_d[:], in_=numT_sb[:])
        nc.sync.dma_start(out=den_d[:], in_=den_sb[:])

    nc.compile()
    return nc


def _get_program(repeat=1):
    key = ("nc", repeat)
    if key not in _prog_cache:
        _prog_cache[key] = _build_program(repeat)
    return _prog_cache[key]


def _make_in_maps(x, adj, W_l, b_l, W_r, b_r, att):
    att_e8, hsel, ident = _host_consts(att)
    shared = {
        "W_l_bf": _bf16(W_l), "W_r_bf": _bf16(W_r),
        "bl_row": np.ascontiguousarray(_bf16(b_l).reshape(1, HC)),
        "br_row": np.ascontiguousarray(_bf16(b_r).reshape(1, HC)),
        "att_e8": att_e8, "hsel": hsel, "ident": ident,
    }
    in_maps = []
    for b in range(B):
        xT, maskel, xlexp = _host_prep_core(b, x, adj, W_l, b_l, att)
        in_maps.append({"xT": xT, "maskel_bf": maskel, "xlexp": xlexp,
                        **shared})
    return in_maps


# ------------------------------------------------------------------- kernel
def kernel(x, adj, W_l, b_l, W_r, b_r, att, bias):
    from concourse.bass_utils import run_bass_kernel_spmd

    x = np.asarray(x, np.float32)
    adj = np.asarray(adj)
    bias = np.asarray(bias, np.float32)

    in_maps = _make_in_maps(x, adj, W_l, b_l, W_r, b_r, att)
    nc = _get_program()
    res = run_bass_kernel_spmd(nc, in_maps, list(range(B)))

    out = np.empty((B, N, OUT_DIM), np.float32)
    for b in range(B):
        numT = np.asarray(res.results[b]["numT"])   # [128, 512]
        den = np.asarray(res.results[b]["den"])     # [4, 512]
        denx = np.repeat(den.T, PER_HEAD, axis=1)   # [512, 128]
        out[b] = numT.T / denx + bias
    return out
